# revision 30
# baseline (speedup 1.0000x reference)
"""Trainium2 Bass kernel for MultiHeadSelfAttention2D.

Problem: x(4,256,64,64); q,k,v,proj 1x1-conv projections; 4 heads x 64 dim;
full 4096x4096 attention per (batch,head); out = gamma*proj + x.

Sharding: 8 cores = batch(4) x query-half(2). Each core computes its full
output slice out[b][:, nhalf] on-device.

Design notes (v1, rewritten from the 592us baseline):
- The PE HAM clock gate only un-throttles (1.2->2.4 GHz) under sustained
  full-width activity, and 64-contraction score matmuls alone never trip it.
  Scores for the two heads of a 128-partition pair are interleaved on PE
  tile rows (64,0)/(0,0), which both overlaps them ~2x and presents
  full-width activity.
- exp evacuation of the score PSUM is the true floor (only ScalarE+DVE can
  read PSUM). The 33.5M exps per core are split between ScalarE (hardware
  Exp -> fp8e4) and DVE (Schraudolph bit-trick exp: one tensor_scalar
  mult+add writing uint8 == fp8e4 bits, saturating at 0 below). Assignment
  is per (head, query-block) unit so each softmax unit sees one engine.
- PV uses fp8e4 DoubleRow (2 key-subchunks packed, 2x throughput), with V
  padded to 128 columns (65-col ldweights fails the ISA checker; padding to
  128 costs nothing since PSUM usage is free-dim bound). A ones-column in V
  accumulates the softmax denominator in the same matmul.
- V bias is folded into the output projection bias (bp' = bp + Wp bv), so
  the V eviction is a pure fp8 cast.
- Normalization: ScalarE hardware Reciprocal (measured 1e-5 accurate) of
  the denominator row, rank-1 ones-matmul broadcast, one DVE multiply.
Host only concatenates the 8 slices.
"""

import numpy as np

import concourse.bass as bass
import concourse.mybir as mybir
import concourse.tile as tile

B, C, H, W, HEADS = 4, 256, 64, 64, 4
HD = C // HEADS  # 64
HW = H * W  # 4096
NHALF = HW // 2  # 2048
QB = 512  # query block
NB = NHALF // QB  # 4
NCH = HW // 256  # 16 key chunks of 256
SCALE = 1.0 / np.sqrt(HD)  # 1/8
SHIFT = 2.0  # exp(s*SCALE - SHIFT): keeps fp8 range safe; cancels in softmax
LOG2E = 1.4426950408889634
# DVE bit-trick: fp8e4(bias7) bits = round(x*TC0 + TC1); uint8 write
# saturates at 0 (underflow -> +0.0) and can't reach the e=15 specials.
TC0 = 8.0 * SCALE * LOG2E
TC1 = 56.0 - 8.0 * SHIFT * LOG2E - 0.375

F32 = mybir.dt.float32
BF16 = mybir.dt.bfloat16
FP8 = mybir.dt.float8e4
U8 = mybir.dt.uint8

# unit u = ((hp*NB + nb)*2 + j): True -> ScalarE exp, False -> DVE trick
ASSIGN = [True, False] * 8


def _fix_tail_drain(nc, keep=1):
    """This walrus build rejects instructions with more than a couple of
    semaphore waits. Inserting a same-engine NoOp immediately before an
    instruction is semantically identical (the engine blocks at the NoOp
    instead), so split any excess waits onto adjacent NoOps."""
    fn = nc.m.functions[0]
    for bi, blk in enumerate(fn.blocks):
        insts = list(blk.instructions)
        changed = False
        new_list = []
        for ins in insts:
            si = ins.sync_info
            if si is not None and len(si.on_wait) > keep:
                waits = list(si.on_wait)
                kept, excess = waits[:keep], waits[keep:]
                for j, w in enumerate(excess):
                    new_list.append(
                        mybir.InstNoOp(
                            name=f"waitfix-{bi}-{ins.name}-{j}",
                            engine=ins.engine,
                            sync_info=mybir.SyncInfo(on_wait=[w], on_update=[]),
                        )
                    )
                ins.sync_info = mybir.SyncInfo(on_wait=kept, on_update=si.on_update)
                changed = True
            new_list.append(ins)
        if changed:
            blk.instructions = new_list


def _recip(nc, out, in_):
    """ScalarE hardware Reciprocal (bass raises on it for accuracy reasons;
    measured max rel err ~1e-5 here, far inside our budget)."""
    imm = lambda v: mybir.ImmediateValue(dtype=mybir.dt.float32, value=float(v))
    return nc.scalar.add_instruction(
        mybir.InstActivation(
            name=nc.get_next_instruction_name(),
            func=mybir.ActivationFunctionType.Reciprocal,
            ins=[nc.scalar.lower_ap(in_), imm(0.0), imm(1.0), imm(0.0)],
            outs=[nc.scalar.lower_ap(out)],
        )
    )


def build(fix=True):
    from concourse.masks import make_identity

    nc = bass.Bass("TRN2", target_bir_lowering=False)

    x_d = nc.dram_tensor("x", [C, HW], F32, kind="ExternalInput")
    xq_d = nc.dram_tensor("xq", [C, NHALF], F32, kind="ExternalInput")
    w_d = {
        n: nc.dram_tensor(n, [C, C], F32, kind="ExternalInput")
        for n in ("wq", "wk", "wv", "wp")
    }
    b_d = {
        n: nc.dram_tensor(n, [C], F32, kind="ExternalInput")
        for n in ("bq", "bk", "bv", "bp")
    }
    gamma_d = nc.dram_tensor("gamma", [1], F32, kind="ExternalInput")
    out_d = nc.dram_tensor("out", [C, NHALF], F32, kind="ExternalOutput")

    x_t = x_d[:, :].rearrange("(t p) m -> t p m", p=128)
    xq_t = xq_d[:, :].rearrange("(t p) n -> t p n", p=128)
    out_t = out_d[:, :].rearrange("(t p) n -> t p n", p=128)

    with tile.TileContext(nc) as tc:
        with tc.tile_pool(name="persist", bufs=1) as pp:
            # ---------- persistent tiles ----------
            x16 = [pp.tile([128, HW], BF16, tag=f"x16_{t}", name=f"x16_{t}") for t in range(2)]
            xq16 = [pp.tile([128, NHALF], BF16, tag=f"xq16_{t}", name=f"xq16_{t}") for t in range(2)]
            xb = [pp.tile([128, NHALF], F32, tag=f"xb_{t}", name=f"xb_{t}") for t in range(2)]
            # per-head K, zero-padded to 128 contraction rows: scores stay
            # full-width so the HAM clock gate can always re-warm mid-stream
            kp = [pp.tile([128, HW], BF16, tag=f"kp_{h}", name=f"kp_{h}") for h in range(HEADS)]
            qp = [pp.tile([128, NHALF], BF16, tag=f"qp_{t}", name=f"qp_{t}") for t in range(2)]
            oh2 = [pp.tile([128, NHALF], BF16, tag=f"oh2_{t}", name=f"oh2_{t}") for t in range(2)]
            vta = pp.tile([128, NCH, 2, HEADS, 128], FP8, tag="vta", name="vta")
            wT = {
                n: [pp.tile([128, C], BF16, tag=f"{n}T_{t}", name=f"{n}T_{t}") for t in range(2)]
                for n in ("wq", "wk", "wv", "wp")
            }
            bqp = [pp.tile([128, 1], F32, tag=f"bqp_{t}", name=f"bqp_{t}") for t in range(2)]
            bkp = [pp.tile([128, 1], F32, tag=f"bkp_{t}", name=f"bkp_{t}") for t in range(2)]
            bpp = [pp.tile([128, 1], F32, tag=f"bpp_{t}", name=f"bpp_{t}") for t in range(2)]
            bv16 = [pp.tile([128, 1], BF16, tag=f"bv16_{t}", name=f"bv16_{t}") for t in range(2)]
            gam = pp.tile([128, 1], F32, tag="gam", name="gam")
            gb = [pp.tile([128, 1], F32, tag=f"gb_{t}", name=f"gb_{t}") for t in range(2)]
            nbias = pp.tile([128, 1], F32, tag="nbias", name="nbias")
            ident = pp.tile([128, 128], F32, tag="ident", name="ident")
            wdum = pp.tile([128, 512], BF16, tag="wdum", name="wdum")
            ones1 = pp.tile([1, 64], BF16, tag="ones1", name="ones1")

            nc.vector.memset(ones1, 1.0)
            nc.vector.memset(nbias, -float(SHIFT))
            nc.vector.memset(wdum, 0.0)
            # big memsets go to the otherwise-idle GpSimd engine: on DVE they
            # serialize ~31us ahead of every cast/copy in the setup phase
            for h in range(HEADS):
                # K lives at the same partition rows as that head's Q in qp
                # (rows 64j..64j+64 for j = h%2); the other half is zeros
                z = 64 * (1 - h % 2)
                nc.gpsimd.memset(kp[h][z : z + 64, :], 0.0)
            nc.gpsimd.memset(vta.bitcast(U8), 0)
            # fp8e4 on TRN is bias-7 (1.0 = 0x38); memset would encode via
            # ml_dtypes' bias-8 grid, so write the bits directly
            nc.gpsimd.memset(vta[:, :, :, :, HD : HD + 1].bitcast(U8), 0x38)
            make_identity(nc, ident)

            # gamma broadcast to all partitions
            g_ap = gamma_d[:]
            nc.sync.dma_start(
                out=gam,
                in_=bass.AP(tensor=g_ap.tensor, offset=g_ap.offset, ap=[[0, 128], [1, 1]]),
            )
            for t in range(2):
                bq_r = b_d["bq"][:].rearrange("(t p one) -> t p one", p=128, one=1)
                bk_r = b_d["bk"][:].rearrange("(t p one) -> t p one", p=128, one=1)
                bv_r = b_d["bv"][:].rearrange("(t p one) -> t p one", p=128, one=1)
                bp_r = b_d["bp"][:].rearrange("(t p one) -> t p one", p=128, one=1)
                nc.sync.dma_start(out=bqp[t], in_=bq_r[t])
                nc.sync.dma_start(out=bkp[t], in_=bk_r[t])
                nc.sync.dma_start(out=bpp[t], in_=bp_r[t])

            # ---------- setup: weights transpose, x load/cast, bias folds ----------
            with (
                tc.tile_pool(name="setup_sb", bufs=2) as sb,
                tc.tile_pool(name="setup_ps", bufs=2, space="PSUM") as sps,
            ):
                # keep the PE busy through the DMA-bound setup so the HAM
                # clock gate reaches full rate before the projections
                wps = sps.tile([128, 512], F32, tag="wps", name="wps")
                for _ in range(24):
                    nc.tensor.matmul(
                        wps, lhsT=wdum[:, 0:128], rhs=wdum, start=True, stop=True
                    )

                for t in range(2):
                    bvf = sb.tile([128, 1], F32, tag="bvf", name="bvf")
                    bv_r = b_d["bv"][:].rearrange("(t p one) -> t p one", p=128, one=1)
                    nc.sync.dma_start(out=bvf, in_=bv_r[t])
                    nc.vector.tensor_copy(out=bv16[t], in_=bvf)

                # weights first (small, gate the transposes), then x (gates
                # K/V projections), then xq (Q projection is quick)
                for name in ("wk", "wv", "wq", "wp"):
                    wn = [sb.tile([128, C], F32, tag=f"wnat{t}", name=f"wnat{t}") for t in range(2)]
                    w_r = w_d[name][:, :].rearrange("(t p) c -> t p c", p=128)
                    for t in range(2):
                        nc.sync.dma_start(out=wn[t], in_=w_r[t])
                    for i in range(2):  # o tile
                        for j in range(2):  # c tile
                            tp = sps.tile([128, 128], F32, tag="wtp", name="wtp")
                            nc.tensor.transpose(
                                tp, wn[i][:, j * 128 : (j + 1) * 128], ident
                            )
                            nc.scalar.copy(
                                out=wT[name][j][:, i * 128 : (i + 1) * 128], in_=tp
                            )
                # bp' = bp + Wp bv (folds the V bias into the output bias);
                # gb = gamma * bp'
                for oc in range(2):
                    bps = sps.tile([128, 1], F32, tag="bps", name="bps")
                    for ci in range(2):
                        nc.tensor.matmul(
                            bps,
                            lhsT=wT["wp"][ci][:, oc * 128 : (oc + 1) * 128],
                            rhs=bv16[ci],
                            start=(ci == 0),
                            stop=(ci == 1),
                        )
                    nc.vector.scalar_tensor_tensor(
                        out=gb[oc],
                        in0=bps,
                        scalar=bpp[oc][:, 0:1],
                        in1=gam,
                        op0=mybir.AluOpType.add,
                        op1=mybir.AluOpType.mult,
                    )


            # ---------- K, V projections, interleaved with the x stream ----------
            with (
                tc.tile_pool(name="proj_ps", bufs=4, space="PSUM") as bps,
                tc.tile_pool(name="proj_sb", bufs=1) as psb,
                tc.tile_pool(name="projx_sb", bufs=3) as pxs,
            ):
                kpair = [psb.tile([128, HW], BF16, tag=f"kpair{t}", name=f"kpair{t}") for t in range(2)]
                QTR = HW // 4
                for q in range(4):
                    qs = slice(q * QTR, (q + 1) * QTR)
                    for t in range(2):
                        xf = pxs.tile([128, QTR], F32, tag="xf", name="xf")
                        nc.sync.dma_start(out=xf, in_=x_t[t][:, qs])
                        if t == 0:
                            nc.scalar.copy(out=x16[t][:, qs], in_=xf)
                        else:
                            nc.vector.tensor_copy(out=x16[t][:, qs], in_=xf)
                    # K for this quarter (2 x 512-col blocks per t)
                    for t in range(2):
                        for mb in range(2 * q, 2 * q + 2):
                            ps = bps.tile([128, 512], F32, tag="pk", name="pk")
                            for ci in range(2):
                                nc.tensor.matmul(
                                    ps,
                                    lhsT=wT["wk"][ci][:, 128 * t : 128 * t + 128],
                                    rhs=x16[ci][:, mb * 512 : (mb + 1) * 512],
                                    start=(ci == 0),
                                    stop=(ci == 1),
                                )
                            nc.scalar.activation(
                                out=kpair[t][:, mb * 512 : (mb + 1) * 512],
                                in_=ps,
                                func=mybir.ActivationFunctionType.Identity,
                                bias=bkp[t],
                            )
                    # V for this quarter (8 key chunks); eviction alternates
                    # engines (bias folded into bp')
                    for mc in range(8 * q, 8 * q + 8):
                        ps = bps.tile([128, C], F32, tag="pv", name="pv")
                        for ci in range(2):
                            nc.tensor.matmul(
                                ps,
                                lhsT=x16[ci][:, mc * 128 : (mc + 1) * 128],
                                rhs=wT["wv"][ci][:, :],
                                start=(ci == 0),
                                stop=(ci == 1),
                            )
                        dst = vta[:, mc // 2, mc % 2, :, 0:HD]
                        src_ = ps.rearrange("p (h d) -> p h d", h=HEADS)
                        if mc % 2 == 0:
                            nc.scalar.copy(out=dst, in_=src_)
                        else:
                            nc.vector.tensor_copy(out=dst, in_=src_)
                # split K head-pairs into the zero-padded per-head tiles
                for t in range(2):
                    for j in range(2):
                        for half in range(2):
                            nc.sync.dma_start(
                                out=kp[2 * t + j][
                                    64 * j : 64 * j + 64,
                                    half * (HW // 2) : (half + 1) * (HW // 2),
                                ],
                                in_=kpair[t][
                                    64 * j : 64 * j + 64,
                                    half * (HW // 2) : (half + 1) * (HW // 2),
                                ],
                            )
                # xq + Q projection (the last inputs the attention needs)
                for t in range(2):
                    nc.sync.dma_start(out=xb[t], in_=xq_t[t])
                    nc.vector.tensor_copy(out=xq16[t], in_=xb[t])
                    # xb = xq + gamma*bp'
                    nc.vector.tensor_scalar_add(out=xb[t], in0=xb[t], scalar1=gb[t])
                for t in range(2):
                    for nb in range(NHALF // 512):
                        ps = bps.tile([128, 512], F32, tag="pk", name="pk")
                        for ci in range(2):
                            nc.tensor.matmul(
                                ps,
                                lhsT=wT["wq"][ci][:, 128 * t : 128 * t + 128],
                                rhs=xq16[ci][:, nb * 512 : (nb + 1) * 512],
                                start=(ci == 0),
                                stop=(ci == 1),
                            )
                        nc.scalar.activation(
                            out=qp[t][:, nb * 512 : (nb + 1) * 512],
                            in_=ps,
                            func=mybir.ActivationFunctionType.Identity,
                            bias=bqp[t],
                        )
            # ---------- attention ----------
            with (
                tc.tile_pool(name="st_ps", bufs=3, space="PSUM") as stp,
                tc.tile_pool(name="o_ps", bufs=2, space="PSUM") as opp,
                tc.tile_pool(name="attn_sb", bufs=4) as asb,
                tc.tile_pool(name="ex_sb", bufs=7) as exs_pool,
            ):
                def emit_exp(unit_scalar, st, ex):
                    if unit_scalar:
                        nc.scalar.activation(
                            out=ex,
                            in_=st,
                            func=mybir.ActivationFunctionType.Exp,
                            scale=float(SCALE),
                            bias=nbias,
                        )
                    else:
                        nc.vector.tensor_scalar(
                            out=ex.bitcast(U8),
                            in0=st,
                            scalar1=float(TC0),
                            scalar2=float(TC1),
                            op0=mybir.AluOpType.mult,
                            op1=mybir.AluOpType.add,
                        )

                # normalize is staged across the next block's first
                # chunks so no engine sees a burst at block boundaries
                def norm_recip(ops, hp, j, nb):
                    rb16 = asb.tile([1, QB], BF16, tag="rb16", name="rb16")
                    _recip(nc, rb16, ops[HD : HD + 1, :])
                    return rb16

                def norm_bc(rb16):
                    bc = stp.tile([64, QB], F32, tag="st", name="bc")
                    nc.tensor.matmul(bc, lhsT=ones1, rhs=rb16, start=True, stop=True)
                    return bc

                def norm_bcs(bc):
                    bcs = asb.tile([64, QB], BF16, tag="bcs", name="bcs")
                    nc.scalar.copy(out=bcs, in_=bc)
                    return bcs

                def norm_mul(bcs, ops, hp, j, nb):
                    nc.vector.tensor_mul(
                        out=oh2[hp][64 * j : 64 * j + 64, nb * QB : (nb + 1) * QB],
                        in0=ops[0:HD, :],
                        in1=bcs,
                    )

                def normalize(ops, hp, j, nb):
                    norm_mul(norm_bcs(norm_bc(norm_recip(ops, hp, j, nb))), ops, hp, j, nb)

                pending = []
                ns = {}
                for hp in range(2):
                    for nb in range(NB):
                        u = (hp * NB + nb) * 2
                        engs = (ASSIGN[u], ASSIGN[u + 1])
                        ops = None
                        pvq = []
                        for mc in range(NCH):
                            # staged normalize of the previous block
                            if pending:
                                if mc == 0:
                                    ns["r0"] = norm_recip(*pending[0])
                                elif mc == 1:
                                    ns["r1"] = norm_recip(*pending[1])
                                    ns["b0"] = norm_bc(ns["r0"])
                                elif mc == 2:
                                    ns["b1"] = norm_bc(ns["r1"])
                                    ns["s0"] = norm_bcs(ns["b0"])
                                elif mc == 3:
                                    ns["s1"] = norm_bcs(ns["b1"])
                                    norm_mul(ns["s0"], *pending[0])
                                elif mc == 4:
                                    norm_mul(ns["s1"], *pending[1])
                                    pending = []
                            nst = [stp.tile([128, 2, QB], F32, tag="st", name=f"st{j}") for j in range(2)]
                            for s in range(2):
                                for j in range(2):
                                    nc.tensor.matmul(
                                        nst[j][:, s, :],
                                        lhsT=kp[2 * hp + j][
                                            :,
                                            mc * 256 + s * 128 : mc * 256 + s * 128 + 128,
                                        ],
                                        rhs=qp[hp][
                                            :,
                                            nb * QB : (nb + 1) * QB,
                                        ],
                                        start=True,
                                        stop=True,
                                    )
                            nex = [exs_pool.tile([128, 2, QB], FP8, tag=f"ex{j}", name=f"ex{j}") for j in range(2)]
                            for j in range(2):
                                emit_exp(engs[j], nst[j], nex[j])
                            pvq.append((mc, nex))
                            # PVs lag the scores; held until the prior block's
                            # normalize muls release the ops buffers
                            if mc >= 5:
                                if ops is None:
                                    ops = [opp.tile([128, QB], F32, tag="o", name=f"ops{j}") for j in range(2)]
                                while len(pvq) > 1:
                                    c, exc = pvq.pop(0)
                                    for j in range(2):
                                        nc.tensor.matmul(
                                            ops[j],
                                            lhsT=vta[:, c, :, 2 * hp + j, :],
                                            rhs=exc[j],
                                            start=(c == 0),
                                            stop=(c == NCH - 1),
                                            perf_mode=mybir.MatmulPerfMode.DoubleRow,
                                        )
                        for c, exc in pvq:
                            for j in range(2):
                                nc.tensor.matmul(
                                    ops[j],
                                    lhsT=vta[:, c, :, 2 * hp + j, :],
                                    rhs=exc[j],
                                    start=(c == 0),
                                    stop=(c == NCH - 1),
                                    perf_mode=mybir.MatmulPerfMode.DoubleRow,
                                )
                        pending = [(ops[0], hp, 0, nb), (ops[1], hp, 1, nb)]
                for args in pending:
                    normalize(*args)

            # ---------- output projection + residual ----------
            with (
                tc.tile_pool(name="proj2_ps", bufs=3, space="PSUM") as pps,
                tc.tile_pool(name="res_sb", bufs=3) as rsb,
            ):
                for oc in range(2):
                    for nbo in range(NHALF // 512):
                        ps = pps.tile([128, 512], F32, tag="pp", name="pp")
                        for hp in range(2):
                            nc.tensor.matmul(
                                ps,
                                lhsT=wT["wp"][hp][:, oc * 128 : (oc + 1) * 128],
                                rhs=oh2[hp][:, nbo * 512 : (nbo + 1) * 512],
                                start=(hp == 0),
                                stop=(hp == 1),
                            )
                        res = rsb.tile([128, 512], F32, tag="res", name="res")
                        nc.vector.scalar_tensor_tensor(
                            out=res,
                            in0=ps,
                            scalar=gam[:, 0:1],
                            in1=xb[oc][:, nbo * 512 : (nbo + 1) * 512],
                            op0=mybir.AluOpType.mult,
                            op1=mybir.AluOpType.add,
                        )
                        nc.sync.dma_start(
                            out=out_t[oc, :, nbo * 512 : (nbo + 1) * 512], in_=res
                        )

    if fix:
        _fix_tail_drain(nc)
    return nc


_NC_CACHE = None


def _get_nc():
    global _NC_CACHE
    if _NC_CACHE is None:
        _NC_CACHE = build()
    return _NC_CACHE


def kernel(x, wq, bq, wk, bk, wv, bv, wp, bp, gamma):
    from concourse.bass_utils import run_bass_kernel_spmd

    nc = _get_nc()
    x = np.ascontiguousarray(np.asarray(x, np.float32)).reshape(B, C, HW)
    common = {
        "wq": np.ascontiguousarray(np.asarray(wq, np.float32)),
        "wk": np.ascontiguousarray(np.asarray(wk, np.float32)),
        "wv": np.ascontiguousarray(np.asarray(wv, np.float32)),
        "wp": np.ascontiguousarray(np.asarray(wp, np.float32)),
        "bq": np.ascontiguousarray(np.asarray(bq, np.float32)),
        "bk": np.ascontiguousarray(np.asarray(bk, np.float32)),
        "bv": np.ascontiguousarray(np.asarray(bv, np.float32)),
        "bp": np.ascontiguousarray(np.asarray(bp, np.float32)),
        "gamma": np.ascontiguousarray(np.asarray(gamma, np.float32)),
    }
    in_maps = []
    for core in range(8):
        b, j = core // 2, core % 2
        m = dict(common)
        m["x"] = np.ascontiguousarray(x[b])
        m["xq"] = np.ascontiguousarray(x[b][:, j * NHALF : (j + 1) * NHALF])
        in_maps.append(m)

    res = run_bass_kernel_spmd(nc, in_maps, core_ids=list(range(8)), trace=False)
    out = np.empty((B, C, HW), np.float32)
    for core in range(8):
        b, j = core // 2, core % 2
        out[b][:, j * NHALF : (j + 1) * NHALF] = res.results[core]["out"]
    return out.reshape(B, C, H, W)


# revision 31
# speedup vs baseline: 1.0337x; 1.0337x over previous
"""Trainium2 Bass kernel for MultiHeadSelfAttention2D.

Problem: x(4,256,64,64); q,k,v,proj 1x1-conv projections; 4 heads x 64 dim;
full 4096x4096 attention per (batch,head); out = gamma*proj + x.

Sharding: 8 cores = batch(4) x query-half(2). Each core computes its full
output slice out[b][:, nhalf] on-device.

Design notes (v1, rewritten from the 592us baseline):
- The PE HAM clock gate only un-throttles (1.2->2.4 GHz) under sustained
  full-width activity, and 64-contraction score matmuls alone never trip it.
  Scores for the two heads of a 128-partition pair are interleaved on PE
  tile rows (64,0)/(0,0), which both overlaps them ~2x and presents
  full-width activity.
- exp evacuation of the score PSUM is the true floor (only ScalarE+DVE can
  read PSUM). The 33.5M exps per core are split between ScalarE (hardware
  Exp -> fp8e4) and DVE (Schraudolph bit-trick exp: one tensor_scalar
  mult+add writing uint8 == fp8e4 bits, saturating at 0 below). Assignment
  is per (head, query-block) unit so each softmax unit sees one engine.
- PV uses fp8e4 DoubleRow (2 key-subchunks packed, 2x throughput), with V
  padded to 128 columns (65-col ldweights fails the ISA checker; padding to
  128 costs nothing since PSUM usage is free-dim bound). A ones-column in V
  accumulates the softmax denominator in the same matmul.
- V bias is folded into the output projection bias (bp' = bp + Wp bv), so
  the V eviction is a pure fp8 cast.
- Normalization: ScalarE hardware Reciprocal (measured 1e-5 accurate) of
  the denominator row, rank-1 ones-matmul broadcast, one DVE multiply.
Host only concatenates the 8 slices.
"""

import numpy as np

import concourse.bass as bass
import concourse.mybir as mybir
import concourse.tile as tile

B, C, H, W, HEADS = 4, 256, 64, 64, 4
HD = C // HEADS  # 64
HW = H * W  # 4096
NHALF = HW // 2  # 2048
QB = 512  # query block
NB = NHALF // QB  # 4
NCH = HW // 256  # 16 key chunks of 256
SCALE = 1.0 / np.sqrt(HD)  # 1/8
SHIFT = 2.0  # exp(s*SCALE - SHIFT): keeps fp8 range safe; cancels in softmax
LOG2E = 1.4426950408889634
# DVE bit-trick: fp8e4(bias7) bits = round(x*TC0 + TC1); uint8 write
# saturates at 0 (underflow -> +0.0) and can't reach the e=15 specials.
TC0 = 8.0 * SCALE * LOG2E
TC1 = 56.0 - 8.0 * SHIFT * LOG2E - 0.375

F32 = mybir.dt.float32
BF16 = mybir.dt.bfloat16
FP8 = mybir.dt.float8e4
U8 = mybir.dt.uint8

# unit u = ((hp*NB + nb)*2 + j): True -> ScalarE exp, False -> DVE trick
ASSIGN = [True, False] * 8


def _fix_tail_drain(nc, keep=1):
    """This walrus build rejects instructions with more than a couple of
    semaphore waits. Inserting a same-engine NoOp immediately before an
    instruction is semantically identical (the engine blocks at the NoOp
    instead), so split any excess waits onto adjacent NoOps."""
    fn = nc.m.functions[0]
    for bi, blk in enumerate(fn.blocks):
        insts = list(blk.instructions)
        changed = False
        new_list = []
        for ins in insts:
            si = ins.sync_info
            if si is not None and len(si.on_wait) > keep:
                waits = list(si.on_wait)
                kept, excess = waits[:keep], waits[keep:]
                for j, w in enumerate(excess):
                    new_list.append(
                        mybir.InstNoOp(
                            name=f"waitfix-{bi}-{ins.name}-{j}",
                            engine=ins.engine,
                            sync_info=mybir.SyncInfo(on_wait=[w], on_update=[]),
                        )
                    )
                ins.sync_info = mybir.SyncInfo(on_wait=kept, on_update=si.on_update)
                changed = True
            new_list.append(ins)
        if changed:
            blk.instructions = new_list


def _recip(nc, out, in_):
    """ScalarE hardware Reciprocal (bass raises on it for accuracy reasons;
    measured max rel err ~1e-5 here, far inside our budget)."""
    imm = lambda v: mybir.ImmediateValue(dtype=mybir.dt.float32, value=float(v))
    return nc.scalar.add_instruction(
        mybir.InstActivation(
            name=nc.get_next_instruction_name(),
            func=mybir.ActivationFunctionType.Reciprocal,
            ins=[nc.scalar.lower_ap(in_), imm(0.0), imm(1.0), imm(0.0)],
            outs=[nc.scalar.lower_ap(out)],
        )
    )


def build(fix=True):
    from concourse.masks import make_identity

    nc = bass.Bass("TRN2", target_bir_lowering=False)

    x_d = nc.dram_tensor("x", [C, HW], F32, kind="ExternalInput")
    xq_d = nc.dram_tensor("xq", [C, NHALF], F32, kind="ExternalInput")
    w_d = {
        n: nc.dram_tensor(n, [C, C], F32, kind="ExternalInput")
        for n in ("wq", "wk", "wv", "wp")
    }
    b_d = {
        n: nc.dram_tensor(n, [C], F32, kind="ExternalInput")
        for n in ("bq", "bk", "bv", "bp")
    }
    gamma_d = nc.dram_tensor("gamma", [1], F32, kind="ExternalInput")
    out_d = nc.dram_tensor("out", [C, NHALF], F32, kind="ExternalOutput")

    x_t = x_d[:, :].rearrange("(t p) m -> t p m", p=128)
    xq_t = xq_d[:, :].rearrange("(t p) n -> t p n", p=128)
    out_t = out_d[:, :].rearrange("(t p) n -> t p n", p=128)

    with tile.TileContext(nc) as tc:
        with tc.tile_pool(name="persist", bufs=1) as pp:
            # ---------- persistent tiles ----------
            x16 = [pp.tile([128, HW], BF16, tag=f"x16_{t}", name=f"x16_{t}") for t in range(2)]
            xq16 = [pp.tile([128, NHALF], BF16, tag=f"xq16_{t}", name=f"xq16_{t}") for t in range(2)]
            xb = [pp.tile([128, NHALF], F32, tag=f"xb_{t}", name=f"xb_{t}") for t in range(2)]
            # per-head K, zero-padded to 128 contraction rows: scores stay
            # full-width so the HAM clock gate can always re-warm mid-stream
            kp = [pp.tile([128, HW], BF16, tag=f"kp_{h}", name=f"kp_{h}") for h in range(HEADS)]
            qp = [pp.tile([128, NHALF], BF16, tag=f"qp_{t}", name=f"qp_{t}") for t in range(2)]
            oh2 = [pp.tile([128, NHALF], BF16, tag=f"oh2_{t}", name=f"oh2_{t}") for t in range(2)]
            vta = pp.tile([128, NCH, 2, HEADS, 128], FP8, tag="vta", name="vta")
            wT = {
                n: [pp.tile([128, C], BF16, tag=f"{n}T_{t}", name=f"{n}T_{t}") for t in range(2)]
                for n in ("wq", "wk", "wv", "wp")
            }
            bqp = [pp.tile([128, 1], F32, tag=f"bqp_{t}", name=f"bqp_{t}") for t in range(2)]
            bkp = [pp.tile([128, 1], F32, tag=f"bkp_{t}", name=f"bkp_{t}") for t in range(2)]
            bpp = [pp.tile([128, 1], F32, tag=f"bpp_{t}", name=f"bpp_{t}") for t in range(2)]
            bv16 = [pp.tile([128, 1], BF16, tag=f"bv16_{t}", name=f"bv16_{t}") for t in range(2)]
            gam = pp.tile([128, 1], F32, tag="gam", name="gam")
            gb = [pp.tile([128, 1], F32, tag=f"gb_{t}", name=f"gb_{t}") for t in range(2)]
            nbias = pp.tile([128, 1], F32, tag="nbias", name="nbias")
            ident = pp.tile([128, 128], F32, tag="ident", name="ident")
            wdum = pp.tile([128, 512], BF16, tag="wdum", name="wdum")
            ones1 = pp.tile([1, 64], BF16, tag="ones1", name="ones1")

            nc.vector.memset(ones1, 1.0)
            nc.vector.memset(nbias, -float(SHIFT))
            nc.vector.memset(wdum, 0.0)
            # big memsets go to the otherwise-idle GpSimd engine: on DVE they
            # serialize ~31us ahead of every cast/copy in the setup phase
            for h in range(HEADS):
                # K lives at the same partition rows as that head's Q in qp
                # (rows 64j..64j+64 for j = h%2); the other half is zeros
                z = 64 * (1 - h % 2)
                nc.gpsimd.memset(kp[h][z : z + 64, :], 0.0)
            nc.gpsimd.memset(vta.bitcast(U8), 0)
            # fp8e4 on TRN is bias-7 (1.0 = 0x38); memset would encode via
            # ml_dtypes' bias-8 grid, so write the bits directly
            nc.gpsimd.memset(vta[:, :, :, :, HD : HD + 1].bitcast(U8), 0x38)
            make_identity(nc, ident)

            # gamma broadcast to all partitions
            g_ap = gamma_d[:]
            nc.sync.dma_start(
                out=gam,
                in_=bass.AP(tensor=g_ap.tensor, offset=g_ap.offset, ap=[[0, 128], [1, 1]]),
            )
            for t in range(2):
                bq_r = b_d["bq"][:].rearrange("(t p one) -> t p one", p=128, one=1)
                bk_r = b_d["bk"][:].rearrange("(t p one) -> t p one", p=128, one=1)
                bv_r = b_d["bv"][:].rearrange("(t p one) -> t p one", p=128, one=1)
                bp_r = b_d["bp"][:].rearrange("(t p one) -> t p one", p=128, one=1)
                nc.sync.dma_start(out=bqp[t], in_=bq_r[t])
                nc.sync.dma_start(out=bkp[t], in_=bk_r[t])
                nc.sync.dma_start(out=bpp[t], in_=bp_r[t])

            # ---------- setup: weights transpose, x load/cast, bias folds ----------
            with (
                tc.tile_pool(name="setup_sb", bufs=2) as sb,
                tc.tile_pool(name="setup_ps", bufs=2, space="PSUM") as sps,
            ):
                # keep the PE busy through the DMA-bound setup so the HAM
                # clock gate reaches full rate before the projections
                wps = sps.tile([128, 512], F32, tag="wps", name="wps")
                for _ in range(24):
                    nc.tensor.matmul(
                        wps, lhsT=wdum[:, 0:128], rhs=wdum, start=True, stop=True
                    )

                for t in range(2):
                    bvf = sb.tile([128, 1], F32, tag="bvf", name="bvf")
                    bv_r = b_d["bv"][:].rearrange("(t p one) -> t p one", p=128, one=1)
                    nc.sync.dma_start(out=bvf, in_=bv_r[t])
                    nc.vector.tensor_copy(out=bv16[t], in_=bvf)

                # weights first (small, gate the transposes), then x (gates
                # K/V projections), then xq (Q projection is quick)
                for name in ("wk", "wv", "wq", "wp"):
                    wn = [sb.tile([128, C], F32, tag=f"wnat{t}", name=f"wnat{t}") for t in range(2)]
                    w_r = w_d[name][:, :].rearrange("(t p) c -> t p c", p=128)
                    for t in range(2):
                        nc.sync.dma_start(out=wn[t], in_=w_r[t])
                    for i in range(2):  # o tile
                        for j in range(2):  # c tile
                            tp = sps.tile([128, 128], F32, tag="wtp", name="wtp")
                            nc.tensor.transpose(
                                tp, wn[i][:, j * 128 : (j + 1) * 128], ident
                            )
                            nc.scalar.copy(
                                out=wT[name][j][:, i * 128 : (i + 1) * 128], in_=tp
                            )
                # bp' = bp + Wp bv (folds the V bias into the output bias);
                # gb = gamma * bp'
                for oc in range(2):
                    bps = sps.tile([128, 1], F32, tag="bps", name="bps")
                    for ci in range(2):
                        nc.tensor.matmul(
                            bps,
                            lhsT=wT["wp"][ci][:, oc * 128 : (oc + 1) * 128],
                            rhs=bv16[ci],
                            start=(ci == 0),
                            stop=(ci == 1),
                        )
                    nc.vector.scalar_tensor_tensor(
                        out=gb[oc],
                        in0=bps,
                        scalar=bpp[oc][:, 0:1],
                        in1=gam,
                        op0=mybir.AluOpType.add,
                        op1=mybir.AluOpType.mult,
                    )


            # ---------- K, V, Q projections ----------
            with (
                tc.tile_pool(name="proj_ps", bufs=4, space="PSUM") as bps,
                tc.tile_pool(name="proj_sb", bufs=1) as psb,
                tc.tile_pool(name="projx_sb", bufs=3) as pxs,
            ):
                # x arrives in quarters so the bf16 cast pipelines behind the
                # DMA; xq is queued after x
                for t in range(2):
                    for q in range(4):
                        qs = slice(q * (HW // 4), (q + 1) * (HW // 4))
                        xf = pxs.tile([128, HW // 4], F32, tag="xf", name="xf")
                        nc.sync.dma_start(out=xf, in_=x_t[t][:, qs])
                        if t == 0:
                            nc.scalar.copy(out=x16[t][:, qs], in_=xf)
                        else:
                            nc.vector.tensor_copy(out=x16[t][:, qs], in_=xf)
                for t in range(2):
                    nc.sync.dma_start(out=xb[t], in_=xq_t[t])
                    nc.vector.tensor_copy(out=xq16[t], in_=xb[t])
                    # xb = xq + gamma*bp'
                    nc.vector.tensor_scalar_add(out=xb[t], in0=xb[t], scalar1=gb[t])

                # contiguous full-width warm burst right before the dense
                # projection stream so it runs at 2.4 GHz
                wps2 = bps.tile([128, 512], F32, tag="pk", name="wps2")
                for _ in range(18):
                    nc.tensor.matmul(
                        wps2, lhsT=wdum[:, 0:128], rhs=wdum, start=True, stop=True
                    )

                kpair = [psb.tile([128, HW], BF16, tag=f"kpair{t}", name=f"kpair{t}") for t in range(2)]
                for t in range(2):
                    for mb in range(HW // 512):
                        ps = bps.tile([128, 512], F32, tag="pk", name="pk")
                        for ci in range(2):
                            nc.tensor.matmul(
                                ps,
                                lhsT=wT["wk"][ci][:, 128 * t : 128 * t + 128],
                                rhs=x16[ci][:, mb * 512 : (mb + 1) * 512],
                                start=(ci == 0),
                                stop=(ci == 1),
                            )
                        nc.scalar.activation(
                            out=kpair[t][:, mb * 512 : (mb + 1) * 512],
                            in_=ps,
                            func=mybir.ActivationFunctionType.Identity,
                            bias=bkp[t],
                        )
                    for j in range(2):
                        for half in range(2):
                            nc.sync.dma_start(
                                out=kp[2 * t + j][
                                    64 * j : 64 * j + 64,
                                    half * (HW // 2) : (half + 1) * (HW // 2),
                                ],
                                in_=kpair[t][
                                    64 * j : 64 * j + 64,
                                    half * (HW // 2) : (half + 1) * (HW // 2),
                                ],
                            )
                for mc in range(HW // 128):
                    ps = bps.tile([128, C], F32, tag="pv", name="pv")
                    for ci in range(2):
                        nc.tensor.matmul(
                            ps,
                            lhsT=x16[ci][:, mc * 128 : (mc + 1) * 128],
                            rhs=wT["wv"][ci][:, :],
                            start=(ci == 0),
                            stop=(ci == 1),
                        )
                    dst = vta[:, mc // 2, mc % 2, :, 0:HD]
                    src_ = ps.rearrange("p (h d) -> p h d", h=HEADS)
                    if mc % 2 == 0:
                        nc.scalar.copy(out=dst, in_=src_)
                    else:
                        nc.vector.tensor_copy(out=dst, in_=src_)
                for t in range(2):
                    for nb in range(NHALF // 512):
                        ps = bps.tile([128, 512], F32, tag="pk", name="pk")
                        for ci in range(2):
                            nc.tensor.matmul(
                                ps,
                                lhsT=wT["wq"][ci][:, 128 * t : 128 * t + 128],
                                rhs=xq16[ci][:, nb * 512 : (nb + 1) * 512],
                                start=(ci == 0),
                                stop=(ci == 1),
                            )
                        nc.scalar.activation(
                            out=qp[t][:, nb * 512 : (nb + 1) * 512],
                            in_=ps,
                            func=mybir.ActivationFunctionType.Identity,
                            bias=bqp[t],
                        )
            # ---------- attention ----------
            with (
                tc.tile_pool(name="st_ps", bufs=3, space="PSUM") as stp,
                tc.tile_pool(name="o_ps", bufs=2, space="PSUM") as opp,
                tc.tile_pool(name="attn_sb", bufs=4) as asb,
                tc.tile_pool(name="ex_sb", bufs=7) as exs_pool,
            ):
                def emit_exp(unit_scalar, st, ex):
                    if unit_scalar:
                        nc.scalar.activation(
                            out=ex,
                            in_=st,
                            func=mybir.ActivationFunctionType.Exp,
                            scale=float(SCALE),
                            bias=nbias,
                        )
                    else:
                        nc.vector.tensor_scalar(
                            out=ex.bitcast(U8),
                            in0=st,
                            scalar1=float(TC0),
                            scalar2=float(TC1),
                            op0=mybir.AluOpType.mult,
                            op1=mybir.AluOpType.add,
                        )

                # normalize is staged across the next block's first
                # chunks so no engine sees a burst at block boundaries
                def norm_recip(ops, hp, j, nb):
                    rb16 = asb.tile([1, QB], BF16, tag="rb16", name="rb16")
                    _recip(nc, rb16, ops[HD : HD + 1, :])
                    return rb16

                def norm_bc(rb16):
                    bc = stp.tile([64, QB], F32, tag="st", name="bc")
                    nc.tensor.matmul(bc, lhsT=ones1, rhs=rb16, start=True, stop=True)
                    return bc

                def norm_bcs(bc):
                    bcs = asb.tile([64, QB], BF16, tag="bcs", name="bcs")
                    nc.scalar.copy(out=bcs, in_=bc)
                    return bcs

                def norm_mul(bcs, ops, hp, j, nb):
                    nc.vector.tensor_mul(
                        out=oh2[hp][64 * j : 64 * j + 64, nb * QB : (nb + 1) * QB],
                        in0=ops[0:HD, :],
                        in1=bcs,
                    )

                def normalize(ops, hp, j, nb):
                    norm_mul(norm_bcs(norm_bc(norm_recip(ops, hp, j, nb))), ops, hp, j, nb)

                pending = []
                ns = {}
                for hp in range(2):
                    for nb in range(NB):
                        u = (hp * NB + nb) * 2
                        engs = (ASSIGN[u], ASSIGN[u + 1])
                        ops = None
                        pvq = []
                        for mc in range(NCH):
                            # staged normalize of the previous block
                            if pending:
                                if mc == 0:
                                    ns["r0"] = norm_recip(*pending[0])
                                    ns["r1"] = norm_recip(*pending[1])
                                elif mc == 1:
                                    ns["b0"] = norm_bc(ns["r0"])
                                    ns["b1"] = norm_bc(ns["r1"])
                                elif mc == 2:
                                    ns["s0"] = norm_bcs(ns["b0"])
                                    ns["s1"] = norm_bcs(ns["b1"])
                                elif mc == 3:
                                    norm_mul(ns["s0"], *pending[0])
                                    norm_mul(ns["s1"], *pending[1])
                                    pending = []
                            nst = [stp.tile([128, 2, QB], F32, tag="st", name=f"st{j}") for j in range(2)]
                            for s in range(2):
                                for j in range(2):
                                    nc.tensor.matmul(
                                        nst[j][:, s, :],
                                        lhsT=kp[2 * hp + j][
                                            :,
                                            mc * 256 + s * 128 : mc * 256 + s * 128 + 128,
                                        ],
                                        rhs=qp[hp][
                                            :,
                                            nb * QB : (nb + 1) * QB,
                                        ],
                                        start=True,
                                        stop=True,
                                    )
                            nex = [exs_pool.tile([128, 2, QB], FP8, tag=f"ex{j}", name=f"ex{j}") for j in range(2)]
                            for j in range(2):
                                emit_exp(engs[j], nst[j], nex[j])
                            pvq.append((mc, nex))
                            # PVs lag the scores; held until the prior block's
                            # normalize muls release the ops buffers
                            if mc >= 4:
                                if ops is None:
                                    ops = [opp.tile([128, QB], F32, tag="o", name=f"ops{j}") for j in range(2)]
                                while len(pvq) > 1:
                                    c, exc = pvq.pop(0)
                                    for j in range(2):
                                        nc.tensor.matmul(
                                            ops[j],
                                            lhsT=vta[:, c, :, 2 * hp + j, :],
                                            rhs=exc[j],
                                            start=(c == 0),
                                            stop=(c == NCH - 1),
                                            perf_mode=mybir.MatmulPerfMode.DoubleRow,
                                        )
                        for c, exc in pvq:
                            for j in range(2):
                                nc.tensor.matmul(
                                    ops[j],
                                    lhsT=vta[:, c, :, 2 * hp + j, :],
                                    rhs=exc[j],
                                    start=(c == 0),
                                    stop=(c == NCH - 1),
                                    perf_mode=mybir.MatmulPerfMode.DoubleRow,
                                )
                        pending = [(ops[0], hp, 0, nb), (ops[1], hp, 1, nb)]
                for args in pending:
                    normalize(*args)

            # ---------- output projection + residual ----------
            with (
                tc.tile_pool(name="proj2_ps", bufs=3, space="PSUM") as pps,
                tc.tile_pool(name="res_sb", bufs=3) as rsb,
            ):
                for oc in range(2):
                    for nbo in range(NHALF // 512):
                        ps = pps.tile([128, 512], F32, tag="pp", name="pp")
                        for hp in range(2):
                            nc.tensor.matmul(
                                ps,
                                lhsT=wT["wp"][hp][:, oc * 128 : (oc + 1) * 128],
                                rhs=oh2[hp][:, nbo * 512 : (nbo + 1) * 512],
                                start=(hp == 0),
                                stop=(hp == 1),
                            )
                        res = rsb.tile([128, 512], F32, tag="res", name="res")
                        nc.vector.scalar_tensor_tensor(
                            out=res,
                            in0=ps,
                            scalar=gam[:, 0:1],
                            in1=xb[oc][:, nbo * 512 : (nbo + 1) * 512],
                            op0=mybir.AluOpType.mult,
                            op1=mybir.AluOpType.add,
                        )
                        nc.sync.dma_start(
                            out=out_t[oc, :, nbo * 512 : (nbo + 1) * 512], in_=res
                        )

    if fix:
        _fix_tail_drain(nc)
    return nc


_NC_CACHE = None


def _get_nc():
    global _NC_CACHE
    if _NC_CACHE is None:
        _NC_CACHE = build()
    return _NC_CACHE


def kernel(x, wq, bq, wk, bk, wv, bv, wp, bp, gamma):
    from concourse.bass_utils import run_bass_kernel_spmd

    nc = _get_nc()
    x = np.ascontiguousarray(np.asarray(x, np.float32)).reshape(B, C, HW)
    common = {
        "wq": np.ascontiguousarray(np.asarray(wq, np.float32)),
        "wk": np.ascontiguousarray(np.asarray(wk, np.float32)),
        "wv": np.ascontiguousarray(np.asarray(wv, np.float32)),
        "wp": np.ascontiguousarray(np.asarray(wp, np.float32)),
        "bq": np.ascontiguousarray(np.asarray(bq, np.float32)),
        "bk": np.ascontiguousarray(np.asarray(bk, np.float32)),
        "bv": np.ascontiguousarray(np.asarray(bv, np.float32)),
        "bp": np.ascontiguousarray(np.asarray(bp, np.float32)),
        "gamma": np.ascontiguousarray(np.asarray(gamma, np.float32)),
    }
    in_maps = []
    for core in range(8):
        b, j = core // 2, core % 2
        m = dict(common)
        m["x"] = np.ascontiguousarray(x[b])
        m["xq"] = np.ascontiguousarray(x[b][:, j * NHALF : (j + 1) * NHALF])
        in_maps.append(m)

    res = run_bass_kernel_spmd(nc, in_maps, core_ids=list(range(8)), trace=False)
    out = np.empty((B, C, HW), np.float32)
    for core in range(8):
        b, j = core // 2, core % 2
        out[b][:, j * NHALF : (j + 1) * NHALF] = res.results[core]["out"]
    return out.reshape(B, C, H, W)
